# revision 1
# baseline (speedup 1.0000x reference)
"""Trainium2 Bass kernel for nn_MirrorDescentLinear.

Reference computation:
    w[o,i] = (e1 - e0) / (1 + e0 + e1)            (softmax(+1) - softmax(-1))
    w *= bf16(scales)[o, i//128]                   (per-group scale)
    w *= mask[o,i]                                 (0/1 int mask)
    y = x @ w.T                                    (f32, [8192,4096]@[4096,4096].T)

Sharding (8 cores): tensor-parallel 4-way on out_features x 2-way on tokens.
Each core computes y[t_half, o_quarter] from logits/scales/mask[o_quarter]
and xT[:, t_half]. The host pre-transposes x once (layout-only) so the
contraction dim I lands on SBUF partitions without any on-chip transpose of x.

Per-core device program:
  phase A (weights): exp on ScalarE; 1/d as exp(-ln d) on ScalarE; e1-e0,
    int-mask multiply, stride-0-broadcast group-scale multiply and recip
    multiply on VectorE; w tiles transposed on TensorE (4 per PSUM bank,
    single-copy evacuation) into resident wT[i, o] float32r tiles, one per
    512-wide i-chunk so phase B can start consuming early chunks.
  phase B (matmul): float32r matmuls (full-rate at N=512, FP22 mantissa)
    accumulating over 32 i-chunks into PSUM; VectorE evacuates, DMA stores y.

Measured on trn2 (single-core loop bench, host I/O excluded): ~740 us/core.
"""

import json
import sys

sys.path.insert(0, "/opt/trn_rl_repo")

import numpy as np

import concourse.bass as bass
import concourse.tile as tile
from concourse import mybir
from concourse.bass_utils import run_bass_kernel_spmd
from concourse.masks import make_identity
from concourse.tile_scheduler import N_PROCS
from concourse.vector_clock import ScopedClock, VectorClock

# ---------------------------------------------------------------------------
# Compatibility patches for the bundled walrus (accepts at most ONE sync wait
# per instruction; rejects any wait on Drain).
# ---------------------------------------------------------------------------


def _drain_and_barrier_split(self, tick_clock, wait_clock):
    g = tick_clock.global_clock
    for p in range(N_PROCS):
        tick = g.peek_next(p) - 1
        if tick <= 0:
            continue
        vc = VectorClock()
        vc.require_at_least(p, tick)
        nop = self.nc.sync.nop(nofuse=True, hint="tail_wait_split")
        wait_clock.add_sem_waits(nop.ins, ScopedClock({None: vc}))

    self.nc.sync.drain()

    self.nc.all_engine_barrier()
    assert self.sems is not None
    popped = self.nc._tile_sem_poison_stack.pop()
    assert popped is self._sem_poison
    self.nc.clear_and_free_semaphores(list(self.sems.allocated().values()))
    self.nc.all_engine_barrier()


_orig_to_json_bytes = bass.Bass.to_json_bytes
_split_ctr = [0]


def _to_json_bytes_split(self):
    raw = _orig_to_json_bytes(self)
    m = json.loads(raw)
    changed = False
    for fn in m.get("functions", []):
        for blk in fn.get("blocks", []):
            insts = blk.get("instructions")
            if not insts:
                continue
            out = []
            for inst in insts:
                si = inst.get("sync_info")
                ow = (si or {}).get("on_wait") or []
                eng = inst.get("engine")
                if len(ow) > 1 and eng:
                    changed = True
                    for w in ow[:-1]:
                        _split_ctr[0] += 1
                        nop = {
                            "engine": eng,
                            "ins": [],
                            "outs": [],
                            "name": f"I-wsplit-{_split_ctr[0]}",
                            "opcode": "NoOp",
                            "sync_info": {"on_update": [], "on_wait": [w]},
                            "text_hint": "wait_split",
                        }
                        if inst.get("debug") is not None:
                            nop["debug"] = inst["debug"]
                        out.append(nop)
                    si["on_wait"] = [ow[-1]]
                out.append(inst)
            blk["instructions"] = out
    return json.dumps(m).encode() if changed else raw


_patched = False


def _install_patches():
    global _patched
    if _patched:
        return
    tile.TileContext._drain_and_barrier = _drain_and_barrier_split
    bass.Bass.to_json_bytes = _to_json_bytes_split
    # Calibrate the scheduler's cost model to measured HW rates: ACT and DVE
    # run slower than the stock model (per-op overheads), which otherwise
    # makes the static PE instruction stream stall on weight-production.
    from concourse.hw_specs import TRN2Spec

    TRN2Spec.CYCLE_T = {
        **TRN2Spec.CYCLE_T,
        mybir.EngineType.DVE: 1e9 / 0.96e9 * 1.4,
        mybir.EngineType.Activation: 1e9 / 1.2e9 * 1.9,
    }
    _patched = True


# ---------------------------------------------------------------------------
# Problem constants (hardcoded per contest rules)
# ---------------------------------------------------------------------------

T_FULL, O_FULL, I_FULL, G = 8192, 4096, 4096, 128
N_OSH, N_TSH = 4, 2  # o-quarters x t-halves = 8 cores
O_SH, T_SH = O_FULL // N_OSH, T_FULL // N_TSH  # 1024, 4096
N_OC = O_SH // 512  # 512-wide output chunks per core (2)
NK = I_FULL // 128  # 32 contraction chunks of 128
N_IC = 8  # i-chunks of 512 in phase A
N_OB = O_SH // 128  # 8 o-blocks per core
N_TT = T_SH // 128  # 32 t-tiles per core

f32 = mybir.dt.float32
f32r = mybir.dt.float32r
i32 = mybir.dt.int32
bf16 = mybir.dt.bfloat16

AF = mybir.ActivationFunctionType
ALU = mybir.AluOpType


def build_program(bench_loop=None, phases=("A", "B")) -> bass.Bass:
    _install_patches()
    nc = bass.Bass()
    if bench_loop is None:
        xT = nc.declare_dram_parameter("xT", [I_FULL, T_SH], f32r, isOutput=False)
        logits = nc.declare_dram_parameter("logits", [O_SH, I_FULL, 2], f32, isOutput=False)
        scales = nc.declare_dram_parameter("scales", [O_SH, I_FULL // G], f32, isOutput=False)
        mask = nc.declare_dram_parameter("mask", [O_SH, I_FULL], i32, isOutput=False)
        y = nc.declare_dram_parameter("y", [T_SH, O_SH], f32, isOutput=True)
    else:
        # timing-bench build: no host I/O of the big tensors, body looped
        dummy = nc.declare_dram_parameter("bench_in", [128, 1], f32, isOutput=False)
        dout = nc.declare_dram_parameter("bench_out", [128, 1], f32, isOutput=True)
        xT = nc.dram_tensor("xT_i", [I_FULL, T_SH], f32r)
        logits = nc.dram_tensor("logits_i", [O_SH, I_FULL, 2], f32)
        scales = nc.dram_tensor("scales_i", [O_SH, I_FULL // G], f32)
        mask = nc.dram_tensor("mask_i", [O_SH, I_FULL], i32)
        y = nc.dram_tensor("y_i", [T_SH, O_SH], f32)

    xT_t = xT.rearrange("(k p) t -> p k t", p=128)  # [128, NK, T_SH]
    scales_t = scales.rearrange("(ob p) g -> p ob g", p=128)  # [128, N_OB, 32]

    with tile.TileContext(nc) as tc:
        with (
            tc.tile_pool(name="persist", bufs=1) as persist,
            tc.tile_pool(name="wt", bufs=1) as wt_pool,
            tc.tile_pool(name="wa", bufs=2) as wa,
            tc.tile_pool(name="xin", bufs=2) as xin,
            tc.tile_pool(name="yout", bufs=2) as yout,
            tc.tile_pool(name="psa", bufs=3, space="PSUM") as psa,
            tc.tile_pool(name="psb", bufs=4, space="PSUM") as psb,
        ):
            ident = persist.tile([128, 128], f32)
            make_identity(nc, ident)

            # scales for all o-blocks, rounded through bf16 once
            s_raw = persist.tile([128, N_OB, 32], f32, tag="sraw")
            nc.sync.dma_start(out=s_raw, in_=scales_t)
            s_bf = persist.tile([128, N_OB, 32], bf16, tag="sbf")
            nc.vector.tensor_copy(out=s_bf, in_=s_raw)
            s_r = persist.tile([128, N_OB, 32], f32, tag="sr")
            nc.vector.tensor_copy(out=s_r, in_=s_bf)

            # resident transposed weights, one tile per 512-wide i-chunk
            # (layout [128 part, 4 k-quarters, O_SH])
            wT = [
                wt_pool.tile([128, 4, O_SH], f32r, tag=f"wT{ic}", name=f"wT{ic}")
                for ic in range(N_IC)
            ]

            if "A" not in phases:
                for ic in range(N_IC):
                    nc.vector.memset(wT[ic].bitcast(f32), 0.0)

            if bench_loop is not None:
                dcp = persist.tile([128, 1], f32, tag="dcp")
                nc.sync.dma_start(out=dcp, in_=dummy[:, :])
                nc.sync.dma_start(out=dout[:, :], in_=dcp)

            import contextlib

            loop_cm = (
                tc.For_i(0, bench_loop, 1)
                if bench_loop is not None
                else contextlib.nullcontext()
            )
            with loop_cm:
                _emit_body(nc, tc, wa, xin, yout, psa, psb, wT, ident, s_r,
                           xT_t, logits, mask, y, phases)

    if bench_loop is not None:
        # tie dummy IO so the program has external IO
        pass
    return nc


def _emit_body(nc, tc, wa, xin, yout, psa, psb, wT, ident, s_r, xT_t, logits, mask, y, phases=("A", "B")):
            # ---- phase A: weights (ic-outer so wT[ic] complete early) ----
            for ic in range(N_IC if "A" in phases else 0):
                for ob in range(N_OB):
                    L = wa.tile([128, 512, 2], f32, tag="L", bufs=4)
                    nc.sync.dma_start(
                        out=L, in_=logits[ob * 128 : (ob + 1) * 128, ic * 512 : (ic + 1) * 512, :]
                    )
                    M = wa.tile([128, 512], i32, tag="M", bufs=4)
                    nc.sync.dma_start(
                        out=M, in_=mask[ob * 128 : (ob + 1) * 128, ic * 512 : (ic + 1) * 512]
                    )
                    # E = exp(logits), in place
                    Lf = L.rearrange("p i s -> p (i s)")
                    nc.scalar.activation(out=Lf, in_=Lf, func=AF.Exp)
                    # D = e0 + 1 + e1
                    D = wa.tile([128, 512], f32, tag="D")
                    nc.vector.scalar_tensor_tensor(
                        out=D, in0=L[:, :, 0], scalar=1.0, in1=L[:, :, 1],
                        op0=ALU.add, op1=ALU.add,
                    )
                    # D <- 1/D via exp(-ln D)  (ScalarE; DVE reciprocal is slow)
                    if "norecip" not in phases:
                        nc.scalar.activation(out=D, in_=D, func=AF.Ln)
                        nc.scalar.activation(out=D, in_=D, func=AF.Exp, scale=-1.0)
                    # N = e1 - e0
                    N = wa.tile([128, 512], f32, tag="N")
                    nc.vector.tensor_tensor(
                        out=N, in0=L[:, :, 1], in1=L[:, :, 0], op=ALU.subtract
                    )
                    # N <- N * mask  (DVE auto-casts the int32 operand)
                    nc.vector.tensor_tensor(out=N, in0=N, in1=M, op=ALU.mult)
                    # N <- N * s_g  (stride-0 broadcast of the 4 group scales)
                    s_sl = s_r[:, ob, ic * 4 : (ic + 1) * 4]
                    s_bc = bass.AP(
                        tensor=s_sl.tensor,
                        offset=s_sl.offset,
                        ap=[s_sl.ap[0], s_sl.ap[1], [0, 128]],
                    )
                    N3 = N.rearrange("p (g c) -> p g c", g=4)
                    nc.vector.tensor_tensor(out=N3, in0=N3, in1=s_bc, op=ALU.mult)
                    # N <- N * (1/D)
                    nc.vector.tensor_tensor(out=N, in0=N, in1=D, op=ALU.mult)
                    # transpose 4x 128x128 blocks into one PSUM bank, then
                    # evacuate all four with a single ScalarE copy
                    pt = psa.tile([128, 512], f32, tag="pt")
                    for q in range(4):
                        nc.tensor.transpose(
                            out=pt[:, q * 128 : (q + 1) * 128],
                            in_=N[:, q * 128 : (q + 1) * 128],
                            identity=ident,
                        )
                    nc.vector.tensor_copy(
                        out=wT[ic][:, :, ob * 128 : (ob + 1) * 128],
                        in_=pt.rearrange("p (q c) -> p q c", q=4),
                    )

            # ---- phase B: y[t, o] = sum_k xT[k,t].T @ wT[k][:, o] ----
            # Accumulation is split into 4 quarter-groups of 8 k-chunks
            # (2 i-chunks each) so PSUM tiles close and recycle as soon as
            # early weight chunks exist -- that lets phase B's matmuls fill
            # the TensorE pipe while later weights are still being built.
            for tt in range(N_TT if "B" in phases else 0):
                xTt = xin.tile([128, NK, 128], f32r, tag="xTt")
                nc.sync.dma_start(
                    out=xTt, in_=xT_t[:, :, tt * 128 : (tt + 1) * 128]
                )
                pbs = [psb.tile([128, 512], f32, tag="pb", name=f"pb{oc}") for oc in range(N_OC)]
                for k in range(NK):
                    ic, q = divmod(k, 4)
                    for oc in range(N_OC):
                        nc.tensor.matmul(
                            out=pbs[oc],
                            lhsT=xTt[:, k, :],
                            rhs=wT[ic][:, q, oc * 512 : (oc + 1) * 512],
                            start=(k == 0),
                            stop=(k == NK - 1),
                        )
                y_sb = yout.tile([128, O_SH], f32, tag="ysb", bufs=3)
                for oc in range(N_OC):
                    nc.vector.tensor_copy(
                        out=y_sb[:, oc * 512 : (oc + 1) * 512], in_=pbs[oc]
                    )
                nc.sync.dma_start(
                    out=y[tt * 128 : (tt + 1) * 128, :], in_=y_sb
                )


_prog = None


def _get_program() -> bass.Bass:
    global _prog
    if _prog is None:
        _prog = build_program()
    return _prog


def kernel(x, logits, scales, mask):
    nc = _get_program()
    x = np.asarray(x, dtype=np.float32)
    logits = np.asarray(logits, dtype=np.float32)
    scales = np.asarray(scales, dtype=np.float32)
    mask_i = np.asarray(mask, dtype=np.int32)

    xT = np.ascontiguousarray(x.T)  # [I, T]
    in_maps = []
    for c in range(8):
        th, oq = divmod(c, N_OSH)
        in_maps.append(
            {
                "xT": np.ascontiguousarray(xT[:, th * T_SH : (th + 1) * T_SH]),
                "logits": np.ascontiguousarray(logits[oq * O_SH : (oq + 1) * O_SH]),
                "scales": np.ascontiguousarray(scales[oq * O_SH : (oq + 1) * O_SH]),
                "mask": np.ascontiguousarray(mask_i[oq * O_SH : (oq + 1) * O_SH]),
            }
        )
    res = run_bass_kernel_spmd(nc, in_maps, core_ids=list(range(8)))
    yf = np.empty((T_FULL, O_FULL), dtype=np.float32)
    for c in range(8):
        th, oq = divmod(c, N_OSH)
        yf[th * T_SH : (th + 1) * T_SH, oq * O_SH : (oq + 1) * O_SH] = res.results[c][
            "y"
        ]
    return yf



# revision 11
# speedup vs baseline: 1.4987x; 1.4987x over previous
"""Trainium2 Bass kernel for nn_MirrorDescentLinear (v3 — 8-way out_features).

Reference computation:
    w[o,i] = (e1 - e0) / (1 + e0 + e1)            (softmax(+1) - softmax(-1))
    w *= bf16(scales)[o, i//128]                   (per-group scale)
    w *= mask[o,i]                                 (0/1 int mask)
    y = x @ w.T                                    (f32, [8192,4096]@[4096,4096].T)

Sharding (8 cores): column-parallel on out_features. Each core builds w for
its 512 out-rows (so the O*I weight math is done exactly once across the
machine) and computes y[:, o_slice] for all 8192 tokens.

Per-core structure (PE-bound by design; matmul floor is ~437us/core):
  Weights are produced OB-MAJOR (128 out-rows at a time, full contraction
  depth) and consumed in the same order:
    sweep alpha: for each of 4 obs, y[o_block, 0:2048] accumulates all 32
      k-chunks from xa (2048 tokens resident in SBUF as fp16);
      out tiles [128 o, 512 t] -> ya (o-major fp16, host transposes back).
    sweep beta: remaining 6144 tokens, token-major with x streamed
      (all weights exist by then); out tiles [128 t, 512 o] -> yb (fp16).
  Weight math per 1024-wide i-chunk (this toolchain rejects all InstISA /
  custom-DVE ops, and DVE's native reciprocal is an 8-cycle iterative
  divide, so 1/d uses a bit-trick seed + one tuned Newton step, max rel
  err 1.7e-3; the correctness gate is 2e-2):
    ACT : E0=exp(l0), E1=exp(l1) (fp16), q=square(y0), alpha-y PSUM evac
    Pool: d=1+e0+e1, t2=d*q, ms=mask*scale_bcast, w=t3*y1  (all as
          scalar_tensor_tensor; Pool's plain tensor_tensor ucode is slower)
    DVE : y0=bitcast(~bits(d))*c0 (seed), y1=c1*y0-t2 (Newton),
          u=E1-E0, t3=u*ms (fp16 2x mode), wT PSUM evac
    PE  : 8x 128x128 fp16 transposes per chunk into resident wT
  The `~bits` op alternates DVE/Pool per chunk to balance the two engines.

Host-side prep is layout/dtype marshaling only: x -> fp16 transposed
(shared by all cores), logits -> bf16, mask -> fp16 {0,1}, scales ->
bf16-rounded fp16 (the rounding the reference itself applies), fp16
outputs assembled/transposed back to f32.
"""

import json
import sys

sys.path.insert(0, "/opt/trn_rl_repo")

import numpy as np

import concourse.bass as bass
import concourse.tile as tile
from concourse import mybir
from concourse.bass_utils import run_bass_kernel_spmd
from concourse.masks import make_identity
from concourse.tile_scheduler import N_PROCS
from concourse.vector_clock import ScopedClock, VectorClock

# ---------------------------------------------------------------------------
# Compatibility patches for the bundled walrus (accepts at most ONE sync wait
# per instruction; rejects any wait on Drain).
# ---------------------------------------------------------------------------


def _drain_and_barrier_split(self, tick_clock, wait_clock):
    g = tick_clock.global_clock
    for p in range(N_PROCS):
        tick = g.peek_next(p) - 1
        if tick <= 0:
            continue
        vc = VectorClock()
        vc.require_at_least(p, tick)
        nop = self.nc.sync.nop(nofuse=True, hint="tail_wait_split")
        wait_clock.add_sem_waits(nop.ins, ScopedClock({None: vc}))

    self.nc.sync.drain()

    self.nc.all_engine_barrier()
    assert self.sems is not None
    popped = self.nc._tile_sem_poison_stack.pop()
    assert popped is self._sem_poison
    self.nc.clear_and_free_semaphores(list(self.sems.allocated().values()))
    self.nc.all_engine_barrier()


_orig_to_json_bytes = bass.Bass.to_json_bytes
_split_ctr = [0]


def _to_json_bytes_split(self):
    raw = _orig_to_json_bytes(self)
    m = json.loads(raw)
    changed = False
    for fn in m.get("functions", []):
        for blk in fn.get("blocks", []):
            insts = blk.get("instructions")
            if not insts:
                continue
            out = []
            for inst in insts:
                si = inst.get("sync_info")
                ow = (si or {}).get("on_wait") or []
                eng = inst.get("engine")
                if len(ow) > 1 and eng:
                    changed = True
                    for w in ow[:-1]:
                        _split_ctr[0] += 1
                        nop = {
                            "engine": eng,
                            "ins": [],
                            "outs": [],
                            "name": f"I-wsplit-{_split_ctr[0]}",
                            "opcode": "NoOp",
                            "sync_info": {"on_update": [], "on_wait": [w]},
                            "text_hint": "wait_split",
                        }
                        if inst.get("debug") is not None:
                            nop["debug"] = inst["debug"]
                        out.append(nop)
                    si["on_wait"] = [ow[-1]]
                out.append(inst)
            blk["instructions"] = out
    return json.dumps(m).encode() if changed else raw


_patched = False


def _install_patches():
    global _patched
    if _patched:
        return
    tile.TileContext._drain_and_barrier = _drain_and_barrier_split
    bass.Bass.to_json_bytes = _to_json_bytes_split
    # Calibrate the scheduler's cost model to measured HW rates: ACT and DVE
    # run slower than the stock model (per-op overheads), which otherwise
    # makes the static PE instruction stream stall on weight-production.
    from concourse.hw_specs import TRN2Spec

    TRN2Spec.CYCLE_T = {
        **TRN2Spec.CYCLE_T,
        mybir.EngineType.DVE: 1e9 / 0.96e9 * 1.4,
        mybir.EngineType.Activation: 1e9 / 1.2e9 * 1.9,
    }
    _patched = True


# ---------------------------------------------------------------------------
# Problem constants (hardcoded per contest rules)
# ---------------------------------------------------------------------------

T_FULL, O_FULL, I_FULL, G = 8192, 4096, 4096, 128
N_OSH = 8  # 8-way shard on out_features
O_SH = O_FULL // N_OSH  # 512
T_SH = T_FULL  # every core sees all tokens
NK = I_FULL // 128  # 32 contraction chunks of 128
N_OB = O_SH // 128  # 4 o-blocks per core
N_IC = 4  # 1024-wide i-chunks per ob in weight production
IC = I_FULL // N_IC  # 1024
TA = 2048  # alpha tokens (x resident in SBUF)
N_TC = TA // 512  # 4
TB = T_SH - TA  # 6144 beta tokens (x streamed)
TBB = 256  # beta x-stream chunk (2 token-tiles)
N_TBB = TB // TBB  # 24

# reciprocal seed/Newton constants (RECIPROCAL_APPROX_FAST's, one iteration,
# algebraically folded:  1/d ~= A*n - (sqB*n)^2 * d,  n = bitcast(~bits(d)))
RCA = -0.4714038
RCSB = 0.23549792

f32 = mybir.dt.float32
f16 = mybir.dt.float16
bf16 = mybir.dt.bfloat16
i32 = mybir.dt.int32

AF = mybir.ActivationFunctionType
ALU = mybir.AluOpType


def build_program(bench_loop=None) -> bass.Bass:
    _install_patches()
    nc = bass.Bass()
    if bench_loop is None:
        xT = nc.declare_dram_parameter("xT", [I_FULL, T_SH], f16, isOutput=False)
        logits = nc.declare_dram_parameter("logits", [O_SH, I_FULL, 2], bf16, isOutput=False)
        maskh = nc.declare_dram_parameter("maskh", [O_SH, I_FULL], mybir.dt.int8, isOutput=False)
        scales = nc.declare_dram_parameter("scales", [O_SH, I_FULL // G], f16, isOutput=False)
        ya = nc.declare_dram_parameter("ya", [O_SH, TA], f16, isOutput=True)
        yb = nc.declare_dram_parameter("yb", [TB, O_SH], f16, isOutput=True)
    else:
        dummy = nc.declare_dram_parameter("bench_in", [128, 1], f32, isOutput=False)
        dout = nc.declare_dram_parameter("bench_out", [128, 1], f32, isOutput=True)
        xT = nc.dram_tensor("xT_i", [I_FULL, T_SH], f16)
        logits = nc.dram_tensor("logits_i", [O_SH, I_FULL, 2], bf16)
        maskh = nc.dram_tensor("maskh_i", [O_SH, I_FULL], mybir.dt.int8)
        scales = nc.dram_tensor("scales_i", [O_SH, I_FULL // G], f16)
        ya = nc.dram_tensor("ya_i", [O_SH, TA], f16)
        yb = nc.dram_tensor("yb_i", [TB, O_SH], f16)

    xT_t = xT.rearrange("(k p) t -> p k t", p=128)  # [128, NK, T_SH]
    scales_t = scales.rearrange("(ob p) g -> p ob g", p=128)  # [128, N_OB, 32]

    with tile.TileContext(nc) as tc:
        with (
            tc.tile_pool(name="persist", bufs=1) as persist,
            tc.tile_pool(name="psa", bufs=2, space="PSUM") as psa,
            tc.tile_pool(name="psb", bufs=4, space="PSUM") as psb,
            tc.tile_pool(name="yev", bufs=1) as yev,
        ):
            ident = persist.tile([128, 128], f16, tag="ident", name="ident")
            make_identity(nc, ident)

            s_r = persist.tile([128, N_OB, 32], f16, tag="sr", name="sr")
            nc.sync.dma_start(out=s_r, in_=scales_t)

            # resident transposed weights [i-part, k, o] fp16
            wT = persist.tile([128, NK, O_SH], f16, tag="wT", name="wT")
            # resident alpha x chunks [i-part, k, 512 t] fp16
            xa = [
                persist.tile([128, NK, 512], f16, tag=f"xa{c}", name=f"xa{c}")
                for c in range(N_TC)
            ]

            if bench_loop is not None:
                dcp = persist.tile([128, 1], f32, tag="dcp", name="dcp")
                nc.sync.dma_start(out=dcp, in_=dummy[:, :])
                nc.sync.dma_start(out=dout[:, :], in_=dcp)

            import contextlib

            loop_cm = (
                tc.For_i(0, bench_loop, 1)
                if bench_loop is not None
                else contextlib.nullcontext()
            )
            with loop_cm:
                # xa DMAs interleave into the production DMA stream (the DMA
                # engines are a serial resource; the first logits tiles must
                # not queue behind 16.8MB of x).
                xa_sched = {(0, 0): 0, (0, 2): 1, (1, 0): 2, (1, 2): 3}

                with tc.tile_pool(name="wa", bufs=1) as wa:
                    # Software-pipelined weight production: stage A of chunk
                    # k overlaps stage B of chunk k-1, so no engine ever
                    # waits mid-stream on a cross-engine hop. PE transposes
                    # of chunk k-1 and one alpha psum-group interleave per
                    # iteration, keeping PE saturated.

                    def stage_a(ob, ic):
                        osl = slice(ob * 128, (ob + 1) * 128)
                        isl = slice(ic * IC, (ic + 1) * IC)
                        L = wa.tile([128, IC, 2], bf16, tag="L", bufs=2, name="L")
                        nc.sync.dma_start(out=L, in_=logits[osl, isl, :])
                        M = wa.tile([128, IC], mybir.dt.int8, tag="M", bufs=2, name="M")
                        nc.sync.dma_start(out=M, in_=maskh[osl, isl])
                        if (ob, ic) in xa_sched:
                            c = xa_sched[(ob, ic)]
                            nc.sync.dma_start(
                                out=xa[c], in_=xT_t[:, :, c * 512 : (c + 1) * 512]
                            )
                        E0 = wa.tile([128, IC], f16, tag="E0", bufs=1, name="E0")
                        nc.scalar.activation(out=E0, in_=L[:, :, 0], func=AF.Exp)
                        E1 = wa.tile([128, IC], f16, tag="E1", bufs=1, name="E1")
                        nc.scalar.activation(out=E1, in_=L[:, :, 1], func=AF.Exp)
                        d = wa.tile([128, IC], f32, tag="D", bufs=2, name="D")
                        nc.vector.scalar_tensor_tensor(
                            out=d, in0=E0, scalar=1.0, in1=E1,
                            op0=ALU.add, op1=ALU.add,
                        )
                        nb = wa.tile([128, IC], f32, tag="NB", bufs=2, name="NB")
                        nc.vector.tensor_scalar(
                            out=nb.bitcast(i32), in0=d.bitcast(i32),
                            scalar1=0, op0=ALU.bitwise_not,
                            scalar2=0, op1=ALU.bypass,
                        )
                        u = wa.tile([128, IC], f16, tag="U", bufs=2, name="U")
                        nc.vector.tensor_tensor(out=u, in0=E1, in1=E0, op=ALU.subtract)
                        return d, nb, u, M

                    def stage_b(ob, ic, d, nb, u, M):
                        # q = (sqB*n)^2 via ACT Square's input scale, then
                        # 1/d ~= A*n - d*q
                        q = wa.tile([128, IC], f32, tag="Q", bufs=1, name="Q")
                        nc.scalar.activation(out=q, in_=nb, func=AF.Square, scale=RCSB)
                        nc.gpsimd.tensor_tensor(out=d, in0=d, in1=q, op=ALU.mult)
                        nc.vector.scalar_tensor_tensor(
                            out=nb, in0=nb, scalar=RCA, in1=d,
                            op0=ALU.mult, op1=ALU.subtract,
                        )
                        s_sl = s_r[:, ob, ic * 8 : (ic + 1) * 8]
                        s_bc = bass.AP(
                            tensor=s_sl.tensor,
                            offset=s_sl.offset,
                            ap=[s_sl.ap[0], s_sl.ap[1], [0, 128]],
                        )
                        ms = wa.tile([128, IC], f16, tag="MS", bufs=1, name="MS")
                        nc.gpsimd.tensor_tensor(
                            out=ms.rearrange("p (g c) -> p g c", g=8),
                            in0=M.rearrange("p (g c) -> p g c", g=8),
                            in1=s_bc, op=ALU.mult,
                        )
                        nc.vector.tensor_tensor(out=u, in0=u, in1=ms, op=ALU.mult)
                        w16 = wa.tile([128, IC], f16, tag="W", bufs=2, name="W")
                        nc.gpsimd.tensor_tensor(out=w16, in0=u, in1=nb, op=ALU.mult)
                        return w16

                    def emit_t(ob, ic, w16):
                        osl = slice(ob * 128, (ob + 1) * 128)
                        pt = psa.tile([128, IC], f16, tag="pt", name="pt")
                        for qq in range(8):
                            nc.tensor.transpose(
                                out=pt[:, qq * 128 : (qq + 1) * 128],
                                in_=w16[:, qq * 128 : (qq + 1) * 128],
                                identity=ident,
                            )
                        nc.vector.tensor_copy(
                            out=wT[:, ic * 8 : (ic + 1) * 8, osl],
                            in_=pt.rearrange("p (q c) -> p q c", q=8),
                        )

                    def emit_alpha_tc(ob, tcn):
                        osl = slice(ob * 128, (ob + 1) * 128)
                        pb = psb.tile([128, 512], f32, tag="pb", name="pb")
                        for k in range(NK):
                            nc.tensor.matmul(
                                out=pb,
                                lhsT=wT[:, k, osl],
                                rhs=xa[tcn][:, k, :],
                                start=(k == 0),
                                stop=(k == NK - 1),
                            )
                        ysb = yev.tile([128, 512], f16, tag="ys", bufs=2, name="ys")
                        nc.scalar.activation(out=ysb, in_=pb, func=AF.Copy)
                        nc.sync.dma_start(
                            out=ya[osl, tcn * 512 : (tcn + 1) * 512], in_=ysb
                        )

                    prev = None
                    for ob in range(N_OB):
                        for ic in range(N_IC):
                            sa = stage_a(ob, ic)
                            if prev is not None:
                                pob, pic, psa_ = prev
                                w16 = stage_b(pob, pic, *psa_)
                                emit_t(pob, pic, w16)
                            if ob >= 1:
                                emit_alpha_tc(ob - 1, ic)
                            prev = (ob, ic, sa)
                    pob, pic, psa_ = prev
                    w16 = stage_b(pob, pic, *psa_)
                    emit_t(pob, pic, w16)
                    for tcn in range(N_TC):
                        emit_alpha_tc(N_OB - 1, tcn)

                # ---- sweep beta: remaining tokens, x streamed ----
                with tc.tile_pool(name="bw", bufs=1) as bw:
                    for tb in range(N_TBB):
                        xb = bw.tile([128, NK, TBB], f16, tag="xb", bufs=2, name="xb")
                        nc.sync.dma_start(
                            out=xb,
                            in_=xT_t[:, :, TA + tb * TBB : TA + (tb + 1) * TBB],
                        )
                        for t2 in range(TBB // 128):
                            pb2 = psb.tile([128, O_SH], f32, tag="pb", name="pb")
                            for k in range(NK):
                                nc.tensor.matmul(
                                    out=pb2,
                                    lhsT=xb[:, k, t2 * 128 : (t2 + 1) * 128],
                                    rhs=wT[:, k, :],
                                    start=(k == 0),
                                    stop=(k == NK - 1),
                                )
                            ysb2 = bw.tile([128, O_SH], f16, tag="ys2", bufs=3, name="ys2")
                            nc.vector.tensor_copy(out=ysb2, in_=pb2)
                            nc.sync.dma_start(
                                out=yb[tb * TBB + t2 * 128 : tb * TBB + (t2 + 1) * 128, :],
                                in_=ysb2,
                            )

    return nc


_prog = None


def _get_program() -> bass.Bass:
    global _prog
    if _prog is None:
        _prog = build_program()
    return _prog


def kernel(x, logits, scales, mask):
    from ml_dtypes import bfloat16 as np_bf16

    nc = _get_program()
    x = np.asarray(x, dtype=np.float32)
    logits = np.asarray(logits, dtype=np.float32)
    scales = np.asarray(scales, dtype=np.float32)
    mask = np.asarray(mask)

    xT = np.ascontiguousarray(x.astype(np.float16).T)  # [I, T] fp16, shared
    lg16 = logits.astype(np_bf16)
    m16 = mask.astype(np.int8)
    s16 = scales.astype(np_bf16).astype(np.float16)

    in_maps = []
    for c in range(8):
        osl = slice(c * O_SH, (c + 1) * O_SH)
        in_maps.append(
            {
                "xT": xT,
                "logits": np.ascontiguousarray(lg16[osl]),
                "maskh": np.ascontiguousarray(m16[osl]),
                "scales": np.ascontiguousarray(s16[osl]),
            }
        )
    res = run_bass_kernel_spmd(nc, in_maps, core_ids=list(range(8)))
    yf = np.empty((T_FULL, O_FULL), dtype=np.float32)
    for c in range(8):
        osl = slice(c * O_SH, (c + 1) * O_SH)
        yf[:TA, osl] = res.results[c]["ya"].T.astype(np.float32)
        yf[TA:, osl] = res.results[c]["yb"].astype(np.float32)
    return yf


# revision 30
# speedup vs baseline: 1.5592x; 1.0404x over previous
"""Trainium2 Bass kernel for nn_MirrorDescentLinear (v4 — 8-way out_features).

Reference computation:
    w[o,i] = (e1 - e0) / (1 + e0 + e1)            (softmax(+1) - softmax(-1))
    w *= bf16(scales)[o, i//128]                   (per-group scale)
    w *= mask[o,i]                                 (0/1 int mask)
    y = x @ w.T                                    (f32, [8192,4096]@[4096,4096].T)

Sharding (8 cores): column-parallel on out_features. Each core builds w for
its 512 out-rows (the O*I weight math is done exactly once across the
machine) and computes y[:, o_slice] for all 8192 tokens.

Per-core structure (PE-bound; the fp16 matmul floor is ~440us/core):
  Weights are produced OB-MAJOR (128 out-rows at a time, full contraction
  depth) and consumed in the same order:
    sweep alpha: for each of 4 obs, y[o_block, 0:2048] accumulates all 32
      k-chunks from xa (2048 tokens resident in SBUF as fp16);
      out tiles [128 o, 512 t] -> ya (o-major fp16, host transposes back).
      Each ob's first two psum groups open early (k0-15 accumulate during
      that ob's own production) so PE work tiles evenly at ~7us/slot.
    sweep beta: remaining 6144 tokens, token-major with x streamed
      (all weights exist by then); out tiles [128 t, 512 o] -> yb (fp16).
  Weight math runs as a THREE-stage software pipeline over 1024-wide
  i-chunks (stages A/B/C work on chunks k/k-1/k-2 in the same slot), so
  the serial cross-engine chain exp->d->seed->square->t2->y1->w never
  bounds the slot cadence; every engine opens each slot with ready inputs.
  This toolchain rejects all InstISA/custom-DVE ops and DVE's native
  reciprocal is an 8-cycle iterative divide, so 1/d uses a bit-trick seed
  plus one constant-folded Newton step (1/d ~= A*n - (sqB*n)^2 * d with
  n = bitcast(~bits(d)), d in bf16; max rel err ~8e-3, gate is 2e-2):
    ACT : E0=exp(l0), E1=exp(l1) (fp16), q=(sqB*n)^2 via Square's input
          scale, half the alpha-y PSUM evacuations
    DVE : d=1+e0+e1 (stt), seed ~bits (int16 2x), y1=A*n-t2 (stt),
          u=E1-E0 and u*=ms (fp16 2x), wT PSUM evac, other alpha-y evacs
    Pool: t2=d*q, ms=mask*scale_bcast, w=u*y1  (plain tensor_tensor;
          Pool cannot run STT/TensorScalar or touch PSUM on this walrus)
    PE  : 8x 128x128 fp16 transposes per chunk into resident wT,
          interleaved between alpha psum groups

Host-side prep is layout/dtype marshaling only: x -> fp16 transposed
(shared by all cores), logits -> bf16, mask -> int8, scales ->
bf16-rounded fp16 (the rounding the reference itself applies), fp16
outputs assembled/transposed back to f32.

Measured: TimelineSim (HW-calibrated cost model, same methodology as the
800714ns baseline figure) 513535 ns; direct For_i loop-bench on trn2 via
bench_hw.py: ~574000 ns/loop differential (N=300 vs N=30, noisy +-15%).
Correctness on hardware: rel err 3.7e-3 vs the f32 reference (gate 2e-2).
"""

import json
import sys

sys.path.insert(0, "/opt/trn_rl_repo")

import numpy as np

import concourse.bass as bass
import concourse.tile as tile
from concourse import mybir
from concourse.bass_utils import run_bass_kernel_spmd
from concourse.masks import make_identity
from concourse.tile_scheduler import N_PROCS
from concourse.vector_clock import ScopedClock, VectorClock

# ---------------------------------------------------------------------------
# Compatibility patches for the bundled walrus (accepts at most ONE sync wait
# per instruction; rejects any wait on Drain).
# ---------------------------------------------------------------------------


def _drain_and_barrier_split(self, tick_clock, wait_clock):
    g = tick_clock.global_clock
    for p in range(N_PROCS):
        tick = g.peek_next(p) - 1
        if tick <= 0:
            continue
        vc = VectorClock()
        vc.require_at_least(p, tick)
        nop = self.nc.sync.nop(nofuse=True, hint="tail_wait_split")
        wait_clock.add_sem_waits(nop.ins, ScopedClock({None: vc}))

    self.nc.sync.drain()

    self.nc.all_engine_barrier()
    assert self.sems is not None
    popped = self.nc._tile_sem_poison_stack.pop()
    assert popped is self._sem_poison
    self.nc.clear_and_free_semaphores(list(self.sems.allocated().values()))
    self.nc.all_engine_barrier()


_orig_to_json_bytes = bass.Bass.to_json_bytes
_split_ctr = [0]


def _to_json_bytes_split(self):
    raw = _orig_to_json_bytes(self)
    m = json.loads(raw)
    changed = False
    for fn in m.get("functions", []):
        for blk in fn.get("blocks", []):
            insts = blk.get("instructions")
            if not insts:
                continue
            out = []
            for inst in insts:
                si = inst.get("sync_info")
                ow = (si or {}).get("on_wait") or []
                eng = inst.get("engine")
                if len(ow) > 1 and eng:
                    changed = True
                    for w in ow[:-1]:
                        _split_ctr[0] += 1
                        nop = {
                            "engine": eng,
                            "ins": [],
                            "outs": [],
                            "name": f"I-wsplit-{_split_ctr[0]}",
                            "opcode": "NoOp",
                            "sync_info": {"on_update": [], "on_wait": [w]},
                            "text_hint": "wait_split",
                        }
                        if inst.get("debug") is not None:
                            nop["debug"] = inst["debug"]
                        out.append(nop)
                    si["on_wait"] = [ow[-1]]
                out.append(inst)
            blk["instructions"] = out
    return json.dumps(m).encode() if changed else raw


_patched = False


def _install_patches():
    global _patched
    if _patched:
        return
    tile.TileContext._drain_and_barrier = _drain_and_barrier_split
    bass.Bass.to_json_bytes = _to_json_bytes_split
    # Calibrate the scheduler's cost model to measured HW rates: ACT and DVE
    # run slower than the stock model (per-op overheads), which otherwise
    # makes the static PE instruction stream stall on weight-production.
    from concourse.hw_specs import TRN2Spec

    TRN2Spec.CYCLE_T = {
        **TRN2Spec.CYCLE_T,
        mybir.EngineType.DVE: 1e9 / 0.96e9 * 1.4,
        mybir.EngineType.Activation: 1e9 / 1.2e9 * 1.9,
    }
    _patched = True


# ---------------------------------------------------------------------------
# Problem constants (hardcoded per contest rules)
# ---------------------------------------------------------------------------

T_FULL, O_FULL, I_FULL, G = 8192, 4096, 4096, 128
N_OSH = 8  # 8-way shard on out_features
O_SH = O_FULL // N_OSH  # 512
T_SH = T_FULL  # every core sees all tokens
NK = I_FULL // 128  # 32 contraction chunks of 128
N_OB = O_SH // 128  # 4 o-blocks per core
N_IC = 4  # 1024-wide i-chunks per ob in weight production
IC = I_FULL // N_IC  # 1024
TA = 2048  # alpha tokens (x resident in SBUF)
N_TC = TA // 512  # 4
TB = T_SH - TA  # 6144 beta tokens (x streamed)
TBB = 256  # beta x-stream chunk (2 token-tiles)
N_TBB = TB // TBB  # 24

# reciprocal seed/Newton constants (RECIPROCAL_APPROX_FAST's, one iteration,
# algebraically folded:  1/d ~= A*n - (sqB*n)^2 * d,  n = bitcast(~bits(d)))
RCA = -0.4714038
RCSB = 0.23549792

f32 = mybir.dt.float32
f16 = mybir.dt.float16
bf16 = mybir.dt.bfloat16
i32 = mybir.dt.int32

AF = mybir.ActivationFunctionType
ALU = mybir.AluOpType


def build_program(bench_loop=None) -> bass.Bass:
    _install_patches()
    nc = bass.Bass()
    if bench_loop is None:
        xT = nc.declare_dram_parameter("xT", [I_FULL, T_SH], f16, isOutput=False)
        logits = nc.declare_dram_parameter("logits", [O_SH, I_FULL, 2], bf16, isOutput=False)
        maskh = nc.declare_dram_parameter("maskh", [O_SH, I_FULL], mybir.dt.int8, isOutput=False)
        scales = nc.declare_dram_parameter("scales", [O_SH, I_FULL // G], f16, isOutput=False)
        ya = nc.declare_dram_parameter("ya", [O_SH, TA], f16, isOutput=True)
        yb = nc.declare_dram_parameter("yb", [TB, O_SH], f16, isOutput=True)
    else:
        dummy = nc.declare_dram_parameter("bench_in", [128, 1], f32, isOutput=False)
        dout = nc.declare_dram_parameter("bench_out", [128, 1], f32, isOutput=True)
        xT = nc.dram_tensor("xT_i", [I_FULL, T_SH], f16)
        logits = nc.dram_tensor("logits_i", [O_SH, I_FULL, 2], bf16)
        maskh = nc.dram_tensor("maskh_i", [O_SH, I_FULL], mybir.dt.int8)
        scales = nc.dram_tensor("scales_i", [O_SH, I_FULL // G], f16)
        ya = nc.dram_tensor("ya_i", [O_SH, TA], f16)
        yb = nc.dram_tensor("yb_i", [TB, O_SH], f16)

    xT_t = xT.rearrange("(k p) t -> p k t", p=128)  # [128, NK, T_SH]
    scales_t = scales.rearrange("(ob p) g -> p ob g", p=128)  # [128, N_OB, 32]

    with tile.TileContext(nc) as tc:
        with (
            tc.tile_pool(name="persist", bufs=1) as persist,
            tc.tile_pool(name="psa", bufs=2, space="PSUM") as psa,
            tc.tile_pool(name="psb", bufs=5, space="PSUM") as psb,
            tc.tile_pool(name="yev", bufs=1) as yev,
        ):
            ident = persist.tile([128, 128], f16, tag="ident", name="ident")
            make_identity(nc, ident)

            s_r = persist.tile([128, N_OB, 32], f16, tag="sr", name="sr")
            nc.sync.dma_start(out=s_r, in_=scales_t)

            # resident transposed weights [i-part, k, o] fp16
            wT = persist.tile([128, NK, O_SH], f16, tag="wT", name="wT")
            # resident alpha x chunks [i-part, k, 512 t] fp16
            xa = [
                persist.tile([128, NK, 512], f16, tag=f"xa{c}", name=f"xa{c}")
                for c in range(N_TC)
            ]

            if bench_loop is not None:
                dcp = persist.tile([128, 1], f32, tag="dcp", name="dcp")
                nc.sync.dma_start(out=dcp, in_=dummy[:, :])
                nc.sync.dma_start(out=dout[:, :], in_=dcp)

            import contextlib

            loop_cm = (
                tc.For_i(0, bench_loop, 1)
                if bench_loop is not None
                else contextlib.nullcontext()
            )
            with loop_cm:
                # xa DMAs interleave into the production DMA stream (the DMA
                # engines are a serial resource; the first logits tiles must
                # not queue behind 16.8MB of x).
                xa_sched = {(0, 0): 0, (0, 2): 1, (0, 3): 2, (1, 1): 3}

                with tc.tile_pool(name="wa", bufs=1) as wa:
                    # Three-stage software pipeline over 1024-wide i-chunks.
                    # Stage A (chunk k), B (chunk k-1), C (chunk k-2) run in
                    # the same slot on different chunks, so every engine
                    # opens each slot with ready inputs and the serial
                    # cross-engine chain q->t2->y1->ms->t3->w never bounds
                    # the cadence (it spans two slots).
                    def stage_a(ob, ic):
                        osl = slice(ob * 128, (ob + 1) * 128)
                        isl = slice(ic * IC, (ic + 1) * IC)
                        L = wa.tile([128, IC, 2], bf16, tag="L", bufs=2, name="L")
                        nc.sync.dma_start(out=L, in_=logits[osl, isl, :])
                        if (ob, ic) in xa_sched:
                            c = xa_sched[(ob, ic)]
                            nc.sync.dma_start(
                                out=xa[c], in_=xT_t[:, :, c * 512 : (c + 1) * 512]
                            )
                        E0 = wa.tile([128, IC], f16, tag="E0", bufs=1, name="E0")
                        nc.scalar.activation(out=E0, in_=L[:, :, 0], func=AF.Exp)
                        E1 = wa.tile([128, IC], f16, tag="E1", bufs=1, name="E1")
                        nc.scalar.activation(out=E1, in_=L[:, :, 1], func=AF.Exp)
                        d = wa.tile([128, IC], bf16, tag="D", bufs=2, name="D")
                        nc.vector.scalar_tensor_tensor(
                            out=d, in0=E0, scalar=1.0, in1=E1,
                            op0=ALU.add, op1=ALU.add,
                        )
                        nb = wa.tile([128, IC], bf16, tag="NB", bufs=2, name="NB")
                        nc.vector.tensor_scalar(
                            out=nb.bitcast(mybir.dt.int16),
                            in0=d.bitcast(mybir.dt.int16),
                            scalar1=0, op0=ALU.bitwise_not,
                            scalar2=0, op1=ALU.bypass,
                        )
                        u = wa.tile([128, IC], f16, tag="U", bufs=3, name="U")
                        nc.vector.tensor_tensor(out=u, in0=E1, in1=E0, op=ALU.subtract)
                        return d, nb, u

                    def stage_b(ob, ic, d, nb):
                        osl = slice(ob * 128, (ob + 1) * 128)
                        isl = slice(ic * IC, (ic + 1) * IC)
                        M = wa.tile([128, IC], mybir.dt.int8, tag="M", bufs=2, name="M")
                        nc.sync.dma_start(out=M, in_=maskh[osl, isl])
                        q = wa.tile([128, IC], bf16, tag="Q", bufs=1, name="Q")
                        nc.scalar.activation(out=q, in_=nb, func=AF.Square, scale=RCSB)
                        nc.gpsimd.tensor_tensor(out=d, in0=d, in1=q, op=ALU.mult)
                        y1 = wa.tile([128, IC], f16, tag="Y1", bufs=2, name="Y1")
                        nc.vector.scalar_tensor_tensor(
                            out=y1, in0=nb, scalar=RCA, in1=d,
                            op0=ALU.mult, op1=ALU.subtract,
                        )
                        return M, y1

                    def stage_c(ob, ic, u, M, y1):
                        s_sl = s_r[:, ob, ic * 8 : (ic + 1) * 8]
                        s_bc = bass.AP(
                            tensor=s_sl.tensor,
                            offset=s_sl.offset,
                            ap=[s_sl.ap[0], s_sl.ap[1], [0, 128]],
                        )
                        ms = wa.tile([128, IC], f16, tag="MS", bufs=1, name="MS")
                        nc.gpsimd.tensor_tensor(
                            out=ms.rearrange("p (g c) -> p g c", g=8),
                            in0=M.rearrange("p (g c) -> p g c", g=8),
                            in1=s_bc, op=ALU.mult,
                        )
                        nc.vector.tensor_tensor(out=u, in0=u, in1=ms, op=ALU.mult)
                        w16 = wa.tile([128, IC], f16, tag="W", bufs=2, name="W")
                        nc.gpsimd.tensor_tensor(out=w16, in0=u, in1=y1, op=ALU.mult)
                        return w16

                    def emit_t(ob, ic, w16):
                        osl = slice(ob * 128, (ob + 1) * 128)
                        pt = psa.tile([128, IC], f16, tag="pt", name="pt")
                        for qq in range(8):
                            nc.tensor.transpose(
                                out=pt[:, qq * 128 : (qq + 1) * 128],
                                in_=w16[:, qq * 128 : (qq + 1) * 128],
                                identity=ident,
                            )
                        nc.vector.tensor_copy(
                            out=wT[:, ic * 8 : (ic + 1) * 8, osl],
                            in_=pt.rearrange("p (q c) -> p q c", q=8),
                        )

                    def alpha_mm(ob, tcn, pb, k0, k1):
                        osl = slice(ob * 128, (ob + 1) * 128)
                        for k in range(k0, k1):
                            nc.tensor.matmul(
                                out=pb,
                                lhsT=wT[:, k, osl],
                                rhs=xa[tcn][:, k, :],
                                start=(k == 0),
                                stop=(k == NK - 1),
                            )

                    evac_q = []
                    part = {}
                    evac_alt = [0]

                    def flush_evacs():
                        while evac_q:
                            ob, tcn, pb = evac_q.pop(0)
                            osl = slice(ob * 128, (ob + 1) * 128)
                            ysb = yev.tile([128, 512], f16, tag="ys", bufs=2, name="ys")
                            evac_alt[0] ^= 1
                            if evac_alt[0]:
                                nc.scalar.activation(out=ysb, in_=pb, func=AF.Copy)
                            else:
                                nc.vector.tensor_copy(out=ysb, in_=pb)
                            nc.sync.dma_start(
                                out=ya[osl, tcn * 512 : (tcn + 1) * 512], in_=ysb
                            )

                    def do_item(item):
                        kind, g = item
                        if kind == "pa0":
                            pb = psb.tile([128, 512], f32, tag="pb", name=f"pbp{g}_0")
                            alpha_mm(g, 0, pb, 0, 16)
                            part[(g, 0)] = pb
                        elif kind == "pa1":
                            pb = psb.tile([128, 512], f32, tag="pb", name=f"pbp{g}_1")
                            alpha_mm(g, 1, pb, 0, 16)
                            part[(g, 1)] = pb
                        elif kind == "clos":
                            for tcn in (0, 1):
                                pb = part.pop((g, tcn))
                                alpha_mm(g, tcn, pb, 16, NK)
                                evac_q.append((g, tcn, pb))
                        else:
                            tcn = int(kind[1])
                            pb = psb.tile([128, 512], f32, tag="pb", name=f"pb{g}_{tcn}")
                            alpha_mm(g, tcn, pb, 0, NK)
                            evac_q.append((g, tcn, pb))

                    # alpha work queue: (item, earliest flat slot, PE us)
                    queue = []
                    for g in range(N_OB):
                        queue.append((("pa0", g), 4 * g + 4, 3.5))
                        queue.append((("pa1", g), 4 * g + 4, 3.5))
                        queue.append((("clos", g), 4 * g + 6, 6.9))
                        queue.append((("f2", g), 4 * g + 6, 7.0))
                        queue.append((("f3", g), 4 * g + 7, 7.0))

                    def pop_alpha(s):
                        budget = 7.2
                        while queue and queue[0][1] <= s and budget >= queue[0][2]:
                            item, _, us = queue.pop(0)
                            do_item(item)
                            budget -= us

                    chunks = [(ob, ic) for ob in range(N_OB) for ic in range(N_IC)]
                    hist = {}
                    for s, (ob, ic) in enumerate(chunks):
                        # C first: Pool/DVE open the slot with 2-slot-old
                        # inputs; then A (exps feed d/not mid-slot); B's
                        # q->t2->y1 chain trails and lands before its
                        # next-slot consumers.
                        if s >= 2:
                            cob, cic = chunks[s - 2]
                            u_, M_, y1_ = hist.pop(s - 2)
                            hist[(s - 2, "w")] = stage_c(cob, cic, u_, M_, y1_)
                        hist[s] = stage_a(ob, ic)
                        if s >= 1:
                            pob, pic = chunks[s - 1]
                            d_, nb_, u_ = hist[s - 1]
                            hist[s - 1] = (u_,) + stage_b(pob, pic, d_, nb_)
                        flush_evacs()
                        pop_alpha(s)
                        if s >= 2:
                            cob, cic = chunks[s - 2]
                            emit_t(cob, cic, hist.pop((s - 2, "w")))
                    # tail: drain the pipeline and the alpha queue
                    for s in (16, 17):
                        cob, cic = chunks[s - 2]
                        u_, M_, y1_ = hist.pop(s - 2)
                        w16 = stage_c(cob, cic, u_, M_, y1_)
                        if s == 16:
                            pob, pic = chunks[15]
                            d_, nb_, u_ = hist[15]
                            hist[15] = (u_,) + stage_b(pob, pic, d_, nb_)
                        flush_evacs()
                        pop_alpha(s)
                        emit_t(cob, cic, w16)
                    while queue:
                        flush_evacs()
                        item, _, _ = queue.pop(0)
                        do_item(item)
                    flush_evacs()

                # ---- sweep beta: remaining tokens, x streamed ----
                with tc.tile_pool(name="bw", bufs=1) as bw:
                    for tb in range(N_TBB):
                        xb = bw.tile([128, NK, TBB], f16, tag="xb", bufs=2, name="xb")
                        nc.sync.dma_start(
                            out=xb,
                            in_=xT_t[:, :, TA + tb * TBB : TA + (tb + 1) * TBB],
                        )
                        for t2 in range(TBB // 128):
                            pb2 = psb.tile([128, O_SH], f32, tag="pb", name="pb")
                            for k in range(NK):
                                nc.tensor.matmul(
                                    out=pb2,
                                    lhsT=xb[:, k, t2 * 128 : (t2 + 1) * 128],
                                    rhs=wT[:, k, :],
                                    start=(k == 0),
                                    stop=(k == NK - 1),
                                )
                            ysb2 = bw.tile([128, O_SH], f16, tag="ys2", bufs=3, name="ys2")
                            nc.vector.tensor_copy(out=ysb2, in_=pb2)
                            nc.sync.dma_start(
                                out=yb[tb * TBB + t2 * 128 : tb * TBB + (t2 + 1) * 128, :],
                                in_=ysb2,
                            )

    return nc


_prog = None


def _get_program() -> bass.Bass:
    global _prog
    if _prog is None:
        _prog = build_program()
    return _prog


def kernel(x, logits, scales, mask):
    from ml_dtypes import bfloat16 as np_bf16

    nc = _get_program()
    x = np.asarray(x, dtype=np.float32)
    logits = np.asarray(logits, dtype=np.float32)
    scales = np.asarray(scales, dtype=np.float32)
    mask = np.asarray(mask)

    xT = np.ascontiguousarray(x.astype(np.float16).T)  # [I, T] fp16, shared
    lg16 = logits.astype(np_bf16)
    m16 = mask.astype(np.int8)
    s16 = scales.astype(np_bf16).astype(np.float16)

    in_maps = []
    for c in range(8):
        osl = slice(c * O_SH, (c + 1) * O_SH)
        in_maps.append(
            {
                "xT": xT,
                "logits": np.ascontiguousarray(lg16[osl]),
                "maskh": np.ascontiguousarray(m16[osl]),
                "scales": np.ascontiguousarray(s16[osl]),
            }
        )
    res = run_bass_kernel_spmd(nc, in_maps, core_ids=list(range(8)))
    yf = np.empty((T_FULL, O_FULL), dtype=np.float32)
    for c in range(8):
        osl = slice(c * O_SH, (c + 1) * O_SH)
        yf[:TA, osl] = res.results[c]["ya"].T.astype(np.float32)
        yf[TA:, osl] = res.results[c]["yb"].astype(np.float32)
    return yf
